# revision 13
# baseline (speedup 1.0000x reference)
"""AcousticFeedbackSim kernel for Trainium2 (8 NeuronCores, batch-sharded).

The reference is a partitioned overlap-save FFT convolution, which equals a
linear convolution of inp (B, T) with rir (32768 taps), truncated to T.
We compute it as a block-Toeplitz matmul:

    out_block[i] = sum_{d=0}^{K} x_block[i-d] @ Md[d]

with Md[d][p, q] = rir[d*N + q - p] (valid taps only), precomputed on host.
x is passed pre-transposed (samples on partitions, blocks on free dim, with
K zero blocks in front of each row) so its 128-block column slices serve as
the matmul stationary operand; the Md slices stream as the moving operand and
PSUM accumulates over (d, contraction-chunk). Output lands in natural layout.
"""

import sys

sys.path.insert(0, "/opt/trn_rl_repo")

from contextlib import ExitStack

import numpy as np

import concourse.bacc as bacc
import concourse.mybir as mybir
import concourse.tile as tile
from concourse.bass_utils import run_bass_kernel_spmd

B, T = 16, 524288
N, K = 512, 64
NB = T // N            # 1024 blocks per batch row
ROWS = 2               # batch rows per core
NCORES = 8
D = K + 1              # 65 block-diagonals
PAD = K                # zero blocks in front of each row of xT
WR = PAD + NB          # xT columns per row
CC = N // 128          # 4 contraction chunks of the 512-sample block dim
ITPR = NB // 128       # 8 block-tiles of 128 per row
GROUPS = ROWS * ITPR   # 16 psum accumulation groups
PASS_G = 8             # psum banks used per pass

F32 = mybir.dt.float32
F32R = mybir.dt.float32r

_CACHE = {}


def _build_md(rir: np.ndarray) -> np.ndarray:
    r = rir.reshape(-1).astype(np.float32)
    p = np.arange(N)[:, None]
    q = np.arange(N)[None, :]
    md = np.zeros((D, N, N), np.float32)
    for d in range(D):
        tau = d * N + q - p
        valid = (tau >= 0) & (tau < K * N)
        md[d][valid] = r[np.clip(tau, 0, K * N - 1)][valid]
    return md


def _build_nc():
    nc = bacc.Bacc("TRN2", target_bir_lowering=False, debug=False)
    xt_ext = nc.declare_dram_parameter("xt", [CC, 128, ROWS * WR], F32R, isOutput=False)
    md_ext = nc.declare_dram_parameter("md", [D, N, N], F32R, isOutput=False)
    y_ext = nc.declare_dram_parameter("y", [ROWS, NB, N], F32, isOutput=True)

    with ExitStack() as ctx:
        tc = ctx.enter_context(tile.TileContext(nc))
        xt_pool = ctx.enter_context(tc.tile_pool(name="xt", bufs=1))
        md_pool = ctx.enter_context(tc.tile_pool(name="mdp", bufs=8))
        out_pool = ctx.enter_context(tc.tile_pool(name="outp", bufs=4))
        psum_pool = ctx.enter_context(tc.tile_pool(name="ps", bufs=8, space="PSUM"))

        # xT[cc]: [128 samples, ROWS * (PAD + NB) blocks], zero-padded front
        xt = [
            xt_pool.tile([128, ROWS * WR], F32R, tag=f"xt{cc}", name=f"xt{cc}")
            for cc in range(CC)
        ]
        for cc in range(CC):
            nc.sync.dma_start(xt[cc][:], xt_ext[cc])

        # main accumulation: two passes of 8 psum groups
        for pz in range(GROUPS // PASS_G):
            psums = [
                psum_pool.tile([128, 512], F32, tag="ps", name=f"acc{pz}_{g}")
                for g in range(PASS_G)
            ]
            for d in range(D):
                for cc in range(CC):
                    mt = md_pool.tile([128, 512], F32R, tag="md", name="mt")
                    nc.sync.dma_start(mt[:], md_ext[d, cc * 128 : (cc + 1) * 128, :])
                    for g in range(PASS_G):
                        gi = pz * PASS_G + g
                        r, bt = divmod(gi, ITPR)
                        col = r * WR + PAD + bt * 128 - d
                        nc.tensor.matmul(
                            psums[g][:],
                            xt[cc][:, col : col + 128],
                            mt[:],
                            start=(d == 0 and cc == 0),
                            stop=(d == D - 1 and cc == CC - 1),
                        )
            for g in range(PASS_G):
                gi = pz * PASS_G + g
                r, bt = divmod(gi, ITPR)
                ot = out_pool.tile([128, 512], F32, tag="out", name="ot")
                nc.scalar.copy(ot[:], psums[g][:])
                nc.sync.dma_start(y_ext[r, bt * 128 : (bt + 1) * 128, :], ot[:])
    nc.compile()
    return nc


def kernel(inp: np.ndarray, rir: np.ndarray, nblk) -> np.ndarray:
    assert inp.shape == (B, T) and int(nblk) == N
    if "nc" not in _CACHE:
        _CACHE["nc"] = _build_nc()
    nc = _CACHE["nc"]
    md = _build_md(np.asarray(rir))
    x = np.asarray(inp, np.float32).reshape(B, NB, N)
    # host-side transpose: xt[core][cc, s, r*WR + PAD + j] = x[core*ROWS + r, j, cc*128 + s]
    xt_all = np.zeros((B, CC, 128, WR), np.float32)
    xs = x.transpose(0, 2, 1).reshape(B, CC, 128, NB)  # [b, cc, s, j]
    xt_all[:, :, :, PAD:] = xs
    xt_all = (
        xt_all.reshape(NCORES, ROWS, CC, 128, WR)
        .transpose(0, 2, 3, 1, 4)
        .reshape(NCORES, CC, 128, ROWS * WR)
    )
    in_maps = [
        {"xt": np.ascontiguousarray(xt_all[c]), "md": md} for c in range(NCORES)
    ]
    res = run_bass_kernel_spmd(nc, in_maps, list(range(NCORES)))
    out = np.concatenate([res.results[c]["y"].reshape(ROWS, T) for c in range(NCORES)])
    return out.astype(np.float32)


# revision 15
# speedup vs baseline: 1.5015x; 1.5015x over previous
"""AcousticFeedbackSim kernel for Trainium2 (8 NeuronCores, batch-sharded).

The reference is a partitioned overlap-save FFT convolution, which equals a
linear convolution of inp (B, T) with rir (32768 taps), truncated to T.
We compute it as a block-Toeplitz matmul:

    out_block[i] = sum_{d=0}^{K} x_block[i-d] @ Md[d]

with Md[d][p, q] = rir[d*N + q - p] (valid taps only), precomputed on host.
x is passed pre-transposed (samples on partitions, blocks on free dim, with
K zero blocks in front of each row) so its 128-block column slices serve as
the matmul stationary operand; the Md slices stream as the moving operand and
PSUM accumulates over (d, contraction-chunk). Output lands in natural layout.
"""

import sys

sys.path.insert(0, "/opt/trn_rl_repo")

from contextlib import ExitStack

import numpy as np

import concourse.bacc as bacc
import concourse.mybir as mybir
import concourse.tile as tile
from concourse.bass_utils import run_bass_kernel_spmd

B, T = 16, 524288
N, K = 512, 64
NB = T // N            # 1024 blocks per batch row
ROWS = 2               # batch rows per core
NCORES = 8
D = K + 1              # 65 block-diagonals
PAD = K                # zero blocks in front of each row of xT
WR = PAD + NB          # xT columns per row
CC = N // 128          # 4 contraction chunks of the 512-sample block dim
ITPR = NB // 128       # 8 block-tiles of 128 per row
GROUPS = ROWS * ITPR   # 16 psum accumulation groups
PASS_G = 8             # psum banks used per pass

F32 = mybir.dt.float32
F32R = mybir.dt.float32r

_CACHE = {}


def _build_md(rir: np.ndarray) -> np.ndarray:
    r = rir.reshape(-1).astype(np.float32)
    key = r.tobytes()
    if _CACHE.get("md_key") == key:
        return _CACHE["md"]
    # Md[d][p, q] = rpad[(N-1) + d*N + q - p], rpad zero-padded on both ends
    rp = np.concatenate([np.zeros(N - 1, np.float32), r, np.zeros(N, np.float32)])
    s = rp.strides[0]
    md = np.lib.stride_tricks.as_strided(
        rp[N - 1 :], shape=(D, N, N), strides=(N * s, -s, s)
    ).copy()
    _CACHE["md_key"], _CACHE["md"] = key, md
    return md


def _build_nc():
    nc = bacc.Bacc("TRN2", target_bir_lowering=False, debug=False)
    xt_ext = nc.declare_dram_parameter("xt", [CC, 128, ROWS * WR], F32R, isOutput=False)
    md_ext = nc.declare_dram_parameter("md", [D, N, N], F32R, isOutput=False)
    y_ext = nc.declare_dram_parameter("y", [ROWS, NB, N], F32, isOutput=True)

    with ExitStack() as ctx:
        tc = ctx.enter_context(tile.TileContext(nc))
        xt_pool = ctx.enter_context(tc.tile_pool(name="xt", bufs=1))
        md_pool = ctx.enter_context(tc.tile_pool(name="mdp", bufs=8))
        out_pool = ctx.enter_context(tc.tile_pool(name="outp", bufs=4))
        psum_pool = ctx.enter_context(tc.tile_pool(name="ps", bufs=8, space="PSUM"))

        # xT[cc]: [128 samples, ROWS * (PAD + NB) blocks], zero-padded front
        xt = [
            xt_pool.tile([128, ROWS * WR], F32R, tag=f"xt{cc}", name=f"xt{cc}")
            for cc in range(CC)
        ]
        for cc in range(CC):
            nc.sync.dma_start(xt[cc][:], xt_ext[cc])

        # main accumulation: two passes of 8 psum groups
        for pz in range(GROUPS // PASS_G):
            psums = [
                psum_pool.tile([128, 512], F32, tag="ps", name=f"acc{pz}_{g}")
                for g in range(PASS_G)
            ]
            for d in range(D):
                for cc in range(CC):
                    mt = md_pool.tile([128, 512], F32R, tag="md", name="mt")
                    nc.sync.dma_start(mt[:], md_ext[d, cc * 128 : (cc + 1) * 128, :])
                    for g in range(PASS_G):
                        gi = pz * PASS_G + g
                        r, bt = divmod(gi, ITPR)
                        col = r * WR + PAD + bt * 128 - d
                        nc.tensor.matmul(
                            psums[g][:],
                            xt[cc][:, col : col + 128],
                            mt[:],
                            start=(d == 0 and cc == 0),
                            stop=(d == D - 1 and cc == CC - 1),
                        )
            for g in range(PASS_G):
                gi = pz * PASS_G + g
                r, bt = divmod(gi, ITPR)
                ot = out_pool.tile([128, 512], F32, tag="out", name="ot")
                nc.scalar.copy(ot[:], psums[g][:])
                nc.sync.dma_start(y_ext[r, bt * 128 : (bt + 1) * 128, :], ot[:])
    nc.compile()
    return nc


def _get_runner(nc):
    """Cached jitted PJRT executable (run_bass_via_pjrt rebuilds it per call)."""
    if "runner" in _CACHE:
        return _CACHE["runner"]
    import jax
    from jax.experimental.shard_map import shard_map
    from jax.sharding import Mesh, PartitionSpec

    from concourse import bass2jax

    bass2jax.install_neuronx_cc_hook()
    in_names, out_names, out_avals, zero_shapes = [], [], [], []
    for alloc in nc.m.functions[0].allocations:
        if not isinstance(alloc, mybir.MemoryLocationSet):
            continue
        name = alloc.memorylocations[0].name
        if alloc.kind == "ExternalInput":
            in_names.append(name)
        elif alloc.kind == "ExternalOutput":
            out_names.append(name)
            shape = tuple(alloc.tensor_shape)
            dtype = mybir.dt.np(alloc.dtype)
            out_avals.append(jax.core.ShapedArray(shape, dtype))
            zero_shapes.append((shape, dtype))
    n_params = len(in_names)
    all_names = tuple(in_names) + tuple(out_names)

    def _body(*args):
        return tuple(
            bass2jax._bass_exec_p.bind(
                *args,
                out_avals=tuple(out_avals),
                in_names=all_names,
                out_names=tuple(out_names),
                lowering_input_output_aliases=(),
                sim_require_finite=True,
                sim_require_nnan=True,
                nc=nc,
            )
        )

    mesh = Mesh(np.asarray(jax.devices()[:NCORES]), ("core",))
    nio = n_params + len(out_names)
    sharded = jax.jit(
        shard_map(
            _body,
            mesh=mesh,
            in_specs=(PartitionSpec("core"),) * nio,
            out_specs=(PartitionSpec("core"),) * len(out_names),
            check_rep=False,
        ),
        donate_argnums=tuple(range(n_params, nio)),
        keep_unused=True,
    )
    _CACHE["runner"] = (sharded, in_names, out_names, out_avals, zero_shapes)
    return _CACHE["runner"]


def _transpose_input(inp: np.ndarray) -> np.ndarray:
    # xt[core*CC + cc, s, r*WR + PAD + j] = inp[core*ROWS + r, j*N + cc*128 + s]
    x = np.asarray(inp, np.float32).reshape(B, NB, N)
    xt_all = np.zeros((B, CC, 128, WR), np.float32)
    xt_all[:, :, :, PAD:] = x.transpose(0, 2, 1).reshape(B, CC, 128, NB)
    return np.ascontiguousarray(
        xt_all.reshape(NCORES, ROWS, CC, 128, WR)
        .transpose(0, 2, 3, 1, 4)
        .reshape(NCORES * CC, 128, ROWS * WR)
    )


def kernel(inp: np.ndarray, rir: np.ndarray, nblk) -> np.ndarray:
    assert inp.shape == (B, T) and int(nblk) == N
    if "nc" not in _CACHE:
        _CACHE["nc"] = _build_nc()
    nc = _CACHE["nc"]
    md = _build_md(np.asarray(rir))
    xt_cat = _transpose_input(inp)
    try:
        sharded, in_names, out_names, out_avals, zero_shapes = _get_runner(nc)
        if "md_cat" not in _CACHE or _CACHE["md_cat_key"] is not _CACHE["md_key"]:
            _CACHE["md_cat"] = np.ascontiguousarray(np.tile(md, (NCORES, 1, 1)))
            _CACHE["md_cat_key"] = _CACHE["md_key"]
        cat = {"xt": xt_cat, "md": _CACHE["md_cat"]}
        concat_in = [cat[nm] for nm in in_names]
        concat_zeros = [
            np.zeros((NCORES * s[0], *s[1:]), dt) for s, dt in zero_shapes
        ]
        out_arrs = sharded(*concat_in, *concat_zeros)
        y = np.asarray(out_arrs[out_names.index("y")])
        return y.reshape(B, T).astype(np.float32)
    except Exception:
        _CACHE.pop("runner", None)
        xt_pc = xt_cat.reshape(NCORES, CC, 128, ROWS * WR)
        in_maps = [{"xt": xt_pc[c], "md": md} for c in range(NCORES)]
        res = run_bass_kernel_spmd(nc, in_maps, list(range(NCORES)))
        out = np.concatenate(
            [res.results[c]["y"].reshape(ROWS, T) for c in range(NCORES)]
        )
        return out.astype(np.float32)


# revision 19
# speedup vs baseline: 1.6245x; 1.0819x over previous
"""AcousticFeedbackSim kernel for Trainium2 (8 NeuronCores, batch-sharded).

The reference is a partitioned overlap-save FFT convolution, which equals a
linear convolution of inp (B, T) with rir (32768 taps), truncated to T.
We compute it as a block-Toeplitz matmul:

    out_block[i] = sum_{d=0}^{K} x_block[i-d] @ Md[d]

with Md[d][p, q] = rir[d*N + q - p] (valid taps only), precomputed on host.
x is passed pre-transposed (samples on partitions, blocks on free dim, with
K zero blocks in front of each row) so its 128-block column slices serve as
the matmul stationary operand; the Md slices stream as the moving operand and
PSUM accumulates over (d, contraction-chunk). Output lands in natural layout.
"""

import sys

sys.path.insert(0, "/opt/trn_rl_repo")

from contextlib import ExitStack

import numpy as np

import concourse.bacc as bacc
import concourse.mybir as mybir
import concourse.tile as tile
from concourse.bass_utils import run_bass_kernel_spmd

B, T = 16, 524288
N, K = 512, 64
NB = T // N            # 1024 blocks per batch row
ROWS = 2               # batch rows per core
NCORES = 8
D = K + 1              # 65 block-diagonals
PAD = K                # zero blocks in front of each row of xT
WR = PAD + NB          # xT columns per row
CC = N // 128          # 4 contraction chunks of the 512-sample block dim
ITPR = NB // 128       # 8 block-tiles of 128 per row
GROUPS = ROWS * ITPR   # 16 psum accumulation groups
PASS_G = 8             # psum banks used per pass

F32 = mybir.dt.float32
F32R = mybir.dt.float32r

_CACHE = {}


def _build_md(rir: np.ndarray) -> np.ndarray:
    r = rir.reshape(-1).astype(np.float32)
    key = r.tobytes()
    if _CACHE.get("md_key") == key:
        return _CACHE["md"]
    # Md[d][p, q] = rpad[(N-1) + d*N + q - p], rpad zero-padded on both ends
    rp = np.concatenate([np.zeros(N - 1, np.float32), r, np.zeros(N, np.float32)])
    s = rp.strides[0]
    md = np.lib.stride_tricks.as_strided(
        rp[N - 1 :], shape=(D, N, N), strides=(N * s, -s, s)
    ).copy()
    _CACHE["md_key"], _CACHE["md"] = key, md
    return md


def _build_nc():
    nc = bacc.Bacc("TRN2", target_bir_lowering=False, debug=False)
    xt_ext = nc.declare_dram_parameter("xt", [CC, 128, ROWS * WR], F32R, isOutput=False)
    md_ext = nc.declare_dram_parameter("md", [D, N, N], F32R, isOutput=False)
    y_ext = nc.declare_dram_parameter("y", [ROWS, NB, N], F32, isOutput=True)

    with ExitStack() as ctx:
        tc = ctx.enter_context(tile.TileContext(nc))
        xt_pool = ctx.enter_context(tc.tile_pool(name="xt", bufs=1))
        md_pool = ctx.enter_context(tc.tile_pool(name="mdp", bufs=8))
        out_pool = ctx.enter_context(tc.tile_pool(name="outp", bufs=4))
        psum_pool = ctx.enter_context(tc.tile_pool(name="ps", bufs=8, space="PSUM"))

        # xT[cc]: [128 samples, ROWS * (PAD + NB) blocks], zero-padded front
        xt = [
            xt_pool.tile([128, ROWS * WR], F32R, tag=f"xt{cc}", name=f"xt{cc}")
            for cc in range(CC)
        ]
        for cc in range(CC):
            nc.sync.dma_start(xt[cc][:], xt_ext[cc])

        # main accumulation: two passes of 8 psum groups
        for pz in range(GROUPS // PASS_G):
            psums = [
                psum_pool.tile([128, 512], F32, tag="ps", name=f"acc{pz}_{g}")
                for g in range(PASS_G)
            ]
            for d in range(D):
                for cc in range(CC):
                    mt = md_pool.tile([128, 512], F32R, tag="md", name="mt")
                    nc.sync.dma_start(mt[:], md_ext[d, cc * 128 : (cc + 1) * 128, :])
                    for g in range(PASS_G):
                        gi = pz * PASS_G + g
                        r, bt = divmod(gi, ITPR)
                        col = r * WR + PAD + bt * 128 - d
                        nc.tensor.matmul(
                            psums[g][:],
                            xt[cc][:, col : col + 128],
                            mt[:],
                            start=(d == 0 and cc == 0),
                            stop=(d == D - 1 and cc == CC - 1),
                        )
            for g in range(PASS_G):
                gi = pz * PASS_G + g
                r, bt = divmod(gi, ITPR)
                ot = out_pool.tile([128, 512], F32, tag="out", name="ot")
                nc.scalar.copy(ot[:], psums[g][:])
                nc.sync.dma_start(y_ext[r, bt * 128 : (bt + 1) * 128, :], ot[:])
    nc.compile()
    return nc


def _get_runner(nc):
    """Cached jitted PJRT executable (run_bass_via_pjrt rebuilds it per call)."""
    if "runner" in _CACHE:
        return _CACHE["runner"]
    import jax
    from jax.experimental.shard_map import shard_map
    from jax.sharding import Mesh, PartitionSpec

    from concourse import bass2jax

    bass2jax.install_neuronx_cc_hook()
    in_names, out_names, out_avals, zero_shapes = [], [], [], []
    for alloc in nc.m.functions[0].allocations:
        if not isinstance(alloc, mybir.MemoryLocationSet):
            continue
        name = alloc.memorylocations[0].name
        if alloc.kind == "ExternalInput":
            in_names.append(name)
        elif alloc.kind == "ExternalOutput":
            out_names.append(name)
            shape = tuple(alloc.tensor_shape)
            dtype = mybir.dt.np(alloc.dtype)
            out_avals.append(jax.core.ShapedArray(shape, dtype))
            zero_shapes.append((shape, dtype))
    n_params = len(in_names)
    all_names = tuple(in_names) + tuple(out_names)

    def _body(*args):
        return tuple(
            bass2jax._bass_exec_p.bind(
                *args,
                out_avals=tuple(out_avals),
                in_names=all_names,
                out_names=tuple(out_names),
                lowering_input_output_aliases=(),
                sim_require_finite=True,
                sim_require_nnan=True,
                nc=nc,
            )
        )

    mesh = Mesh(np.asarray(jax.devices()[:NCORES]), ("core",))
    nio = n_params + len(out_names)
    sharded = jax.jit(
        shard_map(
            _body,
            mesh=mesh,
            in_specs=(PartitionSpec("core"),) * nio,
            out_specs=(PartitionSpec("core"),) * len(out_names),
            check_rep=False,
        ),
        donate_argnums=tuple(range(n_params, nio)),
        keep_unused=True,
    )
    _CACHE["runner"] = (sharded, in_names, out_names, out_avals, zero_shapes)
    return _CACHE["runner"]


def _transpose_input(inp: np.ndarray) -> np.ndarray:
    # xt[core*CC + cc, s, r*WR + PAD + j] = inp[core*ROWS + r, j*N + cc*128 + s]
    x = np.asarray(inp, np.float32).reshape(B, NB, N)
    xt_all = np.zeros((B, CC, 128, WR), np.float32)
    xt_all[:, :, :, PAD:] = x.transpose(0, 2, 1).reshape(B, CC, 128, NB)
    return np.ascontiguousarray(
        xt_all.reshape(NCORES, ROWS, CC, 128, WR)
        .transpose(0, 2, 3, 1, 4)
        .reshape(NCORES * CC, 128, ROWS * WR)
    )


def kernel(inp: np.ndarray, rir: np.ndarray, nblk) -> np.ndarray:
    assert inp.shape == (B, T) and int(nblk) == N
    if "nc" not in _CACHE:
        _CACHE["nc"] = _build_nc()
    nc = _CACHE["nc"]
    md = _build_md(np.asarray(rir))
    xt_cat = _transpose_input(inp)
    try:
        sharded, in_names, out_names, out_avals, zero_shapes = _get_runner(nc)
        if "md_cat" not in _CACHE or _CACHE["md_cat_key"] is not _CACHE["md_key"]:
            _CACHE["md_cat"] = np.ascontiguousarray(np.tile(md, (NCORES, 1, 1)))
            _CACHE["md_cat_key"] = _CACHE["md_key"]
        cat = {"xt": xt_cat, "md": _CACHE["md_cat"]}
        concat_in = [cat[nm] for nm in in_names]
        concat_zeros = [
            np.zeros((NCORES * s[0], *s[1:]), dt) for s, dt in zero_shapes
        ]
        out_arrs = sharded(*concat_in, *concat_zeros)
        y = np.asarray(out_arrs[out_names.index("y")])
        return y.reshape(B, T).astype(np.float32)
    except Exception:
        _CACHE.pop("runner", None)
        xt_pc = xt_cat.reshape(NCORES, CC, 128, ROWS * WR)
        in_maps = [{"xt": xt_pc[c], "md": md} for c in range(NCORES)]
        res = run_bass_kernel_spmd(nc, in_maps, list(range(NCORES)))
        out = np.concatenate(
            [res.results[c]["y"].reshape(ROWS, T) for c in range(NCORES)]
        )
        return out.astype(np.float32)
